# revision 2
# baseline (speedup 1.0000x reference)
"""DeepSeek-V3 MoE routing kernel for Trainium2 (Bass/Tile), 8-core SPMD.

Reference semantics (per token, E=256 experts, G=8 groups of 32):
  scores = sigmoid(logits); swb = scores + bias
  group_score[g] = sum of top-2 of swb within group g
  keep top-4 groups; among kept experts take top-8 by swb
  s = scores * onehot(top8); out_vals = sort_desc(s)/(sum(s)+1e-20)*2.5
  out_idx = indices in descending-s order

Sharding: tokens split evenly across 8 NeuronCores (data parallel),
bias replicated.  Inside a core: tiles of 128 tokens (partition dim) x
256 experts (free dim).
"""

import numpy as np

T_FULL = 131072
E = 256
G = 8
EG = 32
N_CORES = 8
T_CORE = T_FULL // N_CORES
P = 128
NEG = -1.0e30


def build_bass(n_tokens: int, dve_only: bool = False):
    """Build the single-core Bass module processing [n_tokens, 256]."""
    from contextlib import ExitStack

    import concourse.bacc as bacc
    import concourse.mybir as mybir
    import concourse.tile as tile

    f32 = mybir.dt.float32
    A = mybir.AluOpType
    AX = mybir.AxisListType
    AF = mybir.ActivationFunctionType

    assert n_tokens % P == 0
    n_tiles = n_tokens // P

    nc = bacc.Bacc("TRN2", target_bir_lowering=False, debug=False)

    logits_d = nc.dram_tensor("logits", [n_tokens, E], f32, kind="ExternalInput").ap()
    bias_d = nc.dram_tensor("bias", [E], f32, kind="ExternalInput").ap()
    idx_d = nc.dram_tensor("idx", [n_tokens, 8], mybir.dt.int32, kind="ExternalOutput").ap()
    vals_d = nc.dram_tensor("vals", [n_tokens, 8], f32, kind="ExternalOutput").ap()

    with tile.TileContext(nc) as tc, ExitStack() as ctx:
        setup = ctx.enter_context(tc.tile_pool(name="setup", bufs=1))
        big = ctx.enter_context(tc.tile_pool(name="big", bufs=3))
        small = ctx.enter_context(tc.tile_pool(name="small", bufs=4))

        # bias broadcast [128, 256]
        bias_row = setup.tile([1, E], f32)
        nc.sync.dma_start(bias_row[:], bias_d.rearrange("(a b) -> a b", a=1))
        bias_bc = setup.tile([P, E], f32)
        nc.gpsimd.partition_broadcast(bias_bc[:], bias_row[:], channels=P)

        for i in range(n_tiles):
            rows = slice(i * P, (i + 1) * P)

            lg = big.tile([P, E], f32, tag="lg")
            nc.sync.dma_start(lg[:], logits_d[rows, :])

            scores = big.tile([P, E], f32, tag="scores")
            nc.scalar.activation(scores[:], lg[:], AF.Sigmoid)

            swb = big.tile([P, E], f32, tag="swb")
            nc.gpsimd.tensor_add(swb[:], scores[:], bias_bc[:])
            swb3 = swb[:].rearrange("p (g e) -> p g e", g=G)

            m1 = small.tile([P, G], f32, tag="m1")
            nc.vector.tensor_reduce(m1[:], swb3, axis=AX.X, op=A.max)

            swb2 = big.tile([P, E], f32, tag="swb2")
            nc.vector.match_replace(swb2[:], m1[:], swb[:], NEG)

            m2 = small.tile([P, G], f32, tag="m2")
            nc.vector.tensor_reduce(m2[:], swb2[:].rearrange("p (g e) -> p g e", g=G),
                                    axis=AX.X, op=A.max)

            gs = small.tile([P, G], f32, tag="gs")
            nc.vector.tensor_add(gs[:], m1[:], m2[:])

            gm8 = small.tile([P, 8], f32, tag="gm8")
            nc.vector.max(out=gm8[:], in_=gs[:])

            goff = small.tile([P, G], f32, tag="goff")
            nc.vector.tensor_scalar(goff[:], gs[:], gm8[:, 3:4], NEG,
                                    op0=A.is_lt, op1=A.mult)

            swbm = big.tile([P, E], f32, tag="swbm")
            swbm3 = swbm[:].rearrange("p (g e) -> p g e", g=G)
            nc.gpsimd.tensor_add(swbm3, swb3, goff[:].to_broadcast([P, G, EG]))

            v8b = small.tile([P, 8], f32, tag="v8b")
            nc.vector.max(out=v8b[:], in_=swbm[:])

            s = big.tile([P, E], f32, tag="s")
            ssum = small.tile([P, 1], f32, tag="ssum")
            nc.vector.scalar_tensor_tensor(
                out=s[:], in0=swbm[:], scalar=v8b[:, 7:8], in1=scores[:],
                op0=A.is_ge, op1=A.mult, accum_out=ssum[:])

            v8u = small.tile([P, 8], f32, tag="v8u")
            nc.vector.max(out=v8u[:], in_=s[:])

            idx8 = small.tile([P, 8], mybir.dt.uint32, tag="idx8")
            nc.vector.max_index(out=idx8[:], in_max=v8u[:], in_values=s[:])

            den = small.tile([P, 1], f32, tag="den")
            nc.vector.tensor_scalar(den[:], ssum[:], 1.0e-20, None, op0=A.add)
            rec = small.tile([P, 1], f32, tag="rec")
            nc.vector.reciprocal(rec[:], den[:])

            vals8 = small.tile([P, 8], f32, tag="vals8")
            nc.vector.tensor_scalar(vals8[:], v8u[:], rec[:], 2.5,
                                    op0=A.mult, op1=A.mult)

            nc.sync.dma_start(idx_d[rows, :], idx8[:].bitcast(mybir.dt.int32))
            nc.sync.dma_start(vals_d[rows, :], vals8[:])

    nc.compile()
    return nc


_NC_CACHE = {}


def _get_nc(n_tokens: int):
    if n_tokens not in _NC_CACHE:
        _NC_CACHE[n_tokens] = build_bass(n_tokens)
    return _NC_CACHE[n_tokens]


def run_spmd(nc, logits, bias, trace=False):
    from concourse import bass_utils

    n = logits.shape[0] // N_CORES
    in_maps = [
        {"logits": np.ascontiguousarray(logits[c * n:(c + 1) * n]),
         "bias": np.ascontiguousarray(bias)}
        for c in range(N_CORES)
    ]
    res = bass_utils.run_bass_kernel_spmd(nc, in_maps, list(range(N_CORES)),
                                          trace=trace)
    idx = np.concatenate([r["idx"] for r in res.results], axis=0)
    vals = np.concatenate([r["vals"] for r in res.results], axis=0)
    return (idx.astype(np.int32), vals.astype(np.float32)), res


def kernel(logits, e_score_correction_bias):
    logits = np.asarray(logits, dtype=np.float32)
    bias = np.asarray(e_score_correction_bias, dtype=np.float32)
    assert logits.shape == (T_FULL, E)
    nc = _get_nc(T_CORE)
    (idx, vals), _ = run_spmd(nc, logits, bias)
    return idx, vals


# revision 4
# speedup vs baseline: 1.0765x; 1.0765x over previous
"""DeepSeek-V3 MoE routing kernel for Trainium2 (Bass/Tile), 8-core SPMD.

Reference semantics (per token, E=256 experts, G=8 groups of 32):
  scores = sigmoid(logits); swb = scores + bias
  group_score[g] = sum of top-2 of swb within group g
  keep top-4 groups; among kept experts take top-8 by swb
  s = scores * onehot(top8); out_vals = sort_desc(s)/(sum(s)+1e-20)*2.5
  out_idx = indices in descending-s order

Sharding: tokens split evenly across 8 NeuronCores (data parallel),
bias replicated.  Inside a core: tiles of 128 tokens (partition dim) x
256 experts (free dim), processed in groups of TB tiles so elementwise
work batches into wide DVE/Pool/ACT instructions.
"""

import numpy as np

T_FULL = 131072
E = 256
G = 8
EG = 32
N_CORES = 8
T_CORE = T_FULL // N_CORES
P = 128
NEG = -1.0e30
TB = 4  # tiles per batch group


def build_bass(n_tokens: int):
    """Build the single-core Bass module processing [n_tokens, 256]."""
    from contextlib import ExitStack

    import concourse.bacc as bacc
    import concourse.mybir as mybir
    import concourse.tile as tile

    f32 = mybir.dt.float32
    A = mybir.AluOpType
    AX = mybir.AxisListType
    AF = mybir.ActivationFunctionType

    assert n_tokens % (P * TB) == 0
    n_groups = n_tokens // (P * TB)
    W = TB * E  # batched free width

    nc = bacc.Bacc("TRN2", target_bir_lowering=False, debug=False)

    logits_d = nc.dram_tensor("logits", [n_tokens, E], f32, kind="ExternalInput").ap()
    bias_d = nc.dram_tensor("bias", [E], f32, kind="ExternalInput").ap()
    idx_d = nc.dram_tensor("idx", [n_tokens, 8], mybir.dt.int32, kind="ExternalOutput").ap()
    vals_d = nc.dram_tensor("vals", [n_tokens, 8], f32, kind="ExternalOutput").ap()

    with tile.TileContext(nc) as tc, ExitStack() as ctx:
        setup = ctx.enter_context(tc.tile_pool(name="setup", bufs=1))
        big = ctx.enter_context(tc.tile_pool(name="big", bufs=2))
        small = ctx.enter_context(tc.tile_pool(name="small", bufs=3))

        # bias broadcast [128, TB*256] (TB copies along free dim)
        bias_row = setup.tile([1, W], f32)
        b2 = bias_d.rearrange("(a b) -> a b", a=1)
        for j in range(TB):
            nc.sync.dma_start(bias_row[:, j * E:(j + 1) * E], b2)
        bias_bc = setup.tile([P, W], f32)
        nc.gpsimd.partition_broadcast(bias_bc[:], bias_row[:], channels=P)

        for i in range(n_groups):
            rows = slice(i * P * TB, (i + 1) * P * TB)
            # DRAM view: [p, j, e] with token = i*P*TB + j*P + p
            dview = logits_d[rows, :].rearrange("(j p) e -> p j e", p=P)

            lg = big.tile([P, W], f32, tag="lg")
            nc.sync.dma_start(lg[:].rearrange("p (j e) -> p j e", j=TB), dview)

            scores = big.tile([P, W], f32, tag="scores")
            nc.scalar.activation(scores[:], lg[:], AF.Sigmoid)

            swb = big.tile([P, W], f32, tag="swb")
            nc.gpsimd.tensor_add(swb[:], scores[:], bias_bc[:])
            swb4 = swb[:].rearrange("p (j g e) -> p j g e", j=TB, g=G)

            m1 = small.tile([P, TB * G], f32, tag="m1")
            nc.vector.tensor_reduce(m1[:].rearrange("p (j g) -> p j g", j=TB),
                                    swb4, axis=AX.X, op=A.max)

            swb2 = big.tile([P, W], f32, tag="swb2")
            for j in range(TB):
                nc.vector.match_replace(
                    out=swb2[:, j * E:(j + 1) * E],
                    in_to_replace=m1[:, j * G:(j + 1) * G],
                    in_values=swb[:, j * E:(j + 1) * E],
                    imm_value=NEG)

            m2 = small.tile([P, TB * G], f32, tag="m2")
            nc.vector.tensor_reduce(
                m2[:].rearrange("p (j g) -> p j g", j=TB),
                swb2[:].rearrange("p (j g e) -> p j g e", j=TB, g=G),
                axis=AX.X, op=A.max)

            gs = small.tile([P, TB * G], f32, tag="gs")
            nc.vector.tensor_add(gs[:], m1[:], m2[:])

            gm8 = small.tile([P, TB * 8], f32, tag="gm8")
            for j in range(TB):
                nc.vector.max(out=gm8[:, j * 8:(j + 1) * 8],
                              in_=gs[:, j * G:(j + 1) * G])

            # cmp = 1.0 where group NOT selected (gs < 4th-largest)
            tg = gm8[:].rearrange("p (j k) -> p j k", j=TB)[:, :, 3]  # [P, TB]
            cmp = small.tile([P, TB * G], f32, tag="cmp")
            nc.vector.tensor_tensor(
                out=cmp[:].rearrange("p (j g) -> p j g", j=TB),
                in0=gs[:].rearrange("p (j g) -> p j g", j=TB),
                in1=tg.to_broadcast([P, TB, G]),
                op=A.is_lt)

            # goff = cmp * NEG; swbm = swb + goff (masked groups -> -1e30)
            goff = small.tile([P, TB * G], f32, tag="goff")
            nc.vector.tensor_scalar(goff[:], cmp[:], NEG, None, op0=A.mult)
            swbm = big.tile([P, W], f32, tag="swbm")
            nc.gpsimd.tensor_add(
                swbm[:].rearrange("p (j g e) -> p j g e", j=TB, g=G),
                swb4,
                goff[:].rearrange("p (j g) -> p j g", j=TB).to_broadcast([P, TB, G, EG]))

            v8b = small.tile([P, TB * 8], f32, tag="v8b")
            for j in range(TB):
                nc.vector.max(out=v8b[:, j * 8:(j + 1) * 8],
                              in_=swbm[:, j * E:(j + 1) * E])

            s = big.tile([P, W], f32, tag="s")
            for j in range(TB):
                nc.vector.scalar_tensor_tensor(
                    out=s[:, j * E:(j + 1) * E],
                    in0=swbm[:, j * E:(j + 1) * E],
                    scalar=v8b[:, j * 8 + 7:j * 8 + 8],
                    in1=scores[:, j * E:(j + 1) * E],
                    op0=A.is_ge, op1=A.mult)

            v8u = small.tile([P, TB * 8], f32, tag="v8u")
            for j in range(TB):
                nc.vector.max(out=v8u[:, j * 8:(j + 1) * 8],
                              in_=s[:, j * E:(j + 1) * E])

            idx8 = small.tile([P, TB * 8], mybir.dt.uint32, tag="idx8")
            for j in range(TB):
                nc.vector.max_index(out=idx8[:, j * 8:(j + 1) * 8],
                                    in_max=v8u[:, j * 8:(j + 1) * 8],
                                    in_values=s[:, j * E:(j + 1) * E])

            ssum = small.tile([P, TB], f32, tag="ssum")
            nc.vector.tensor_reduce(ssum[:],
                                    v8u[:].rearrange("p (j k) -> p j k", j=TB),
                                    axis=AX.X, op=A.add)

            rec = small.tile([P, TB], f32, tag="rec")
            nc.vector.reciprocal(rec[:], ssum[:])

            vals8 = small.tile([P, TB * 8], f32, tag="vals8")
            nc.vector.scalar_tensor_tensor(
                out=vals8[:].rearrange("p (j k) -> p j k", j=TB),
                in0=v8u[:].rearrange("p (j k) -> p j k", j=TB),
                scalar=2.5,
                in1=rec[:].to_broadcast([P, TB, 8]),
                op0=A.mult, op1=A.mult)

            oi = idx_d[rows, :].rearrange("(j p) k -> p j k", p=P)
            ov = vals_d[rows, :].rearrange("(j p) k -> p j k", p=P)
            nc.sync.dma_start(
                oi, idx8[:].bitcast(mybir.dt.int32).rearrange("p (j k) -> p j k", j=TB))
            nc.sync.dma_start(ov, vals8[:].rearrange("p (j k) -> p j k", j=TB))

    nc.compile()
    return nc


_NC_CACHE = {}


def _get_nc(n_tokens: int):
    if n_tokens not in _NC_CACHE:
        _NC_CACHE[n_tokens] = build_bass(n_tokens)
    return _NC_CACHE[n_tokens]


def run_spmd(nc, logits, bias, trace=False):
    from concourse import bass_utils

    n = logits.shape[0] // N_CORES
    in_maps = [
        {"logits": np.ascontiguousarray(logits[c * n:(c + 1) * n]),
         "bias": np.ascontiguousarray(bias)}
        for c in range(N_CORES)
    ]
    res = bass_utils.run_bass_kernel_spmd(nc, in_maps, list(range(N_CORES)),
                                          trace=trace)
    idx = np.concatenate([r["idx"] for r in res.results], axis=0)
    vals = np.concatenate([r["vals"] for r in res.results], axis=0)
    return (idx.astype(np.int32), vals.astype(np.float32)), res


def kernel(logits, e_score_correction_bias):
    logits = np.asarray(logits, dtype=np.float32)
    bias = np.asarray(e_score_correction_bias, dtype=np.float32)
    assert logits.shape == (T_FULL, E)
    nc = _get_nc(T_CORE)
    (idx, vals), _ = run_spmd(nc, logits, bias)
    return idx, vals
